# revision 7
# baseline (speedup 1.0000x reference)
"""MoLE layer (frozen base linear + top-1 routed LoRA experts) on 8 TRN2 cores.

Strategy: data-parallel over tokens (16384 tokens -> 2048/core), all weights
replicated. Per core, per 128-token tile:
  - DMA x tile [128, 2048], PE-transpose to xT (both an fp32r copy for the
    base matmul and an exact-f32 copy for router logits / LoRA down-proj)
  - base:   psum[t, o]  += xT_r[k].T @ W_baseT[k, o]    (fp32r, N=512, full rate)
  - logits: psum[t, 8]  += xT_f[k].T @ router_WT[k]     (fp32 exact: argmax safety)
  - h:      psum[t, er] += xT_f[k].T @ A_allT[k]        (fp32 exact, er = E*R = 128)
  - mask h by top-1 expert (rowmax + is_ge -> one-hot, broadcast over r)
  - delta:  psum[t, o]  += hT_r.T @ B_all[er, o]        (fp32r; SCALING folded in)
  - out = psum + bias  (DVE), DMA out
No collectives needed.
"""

import numpy as np

import concourse.mybir as mybir
import concourse.tile as tile
from concourse import bacc, bass_utils

f32 = mybir.dt.float32
f32r = mybir.dt.float32r

B_, S, D, OUT, E, R = 4, 4096, 2048, 2048, 8, 16
SCALING = 32.0 / 16.0
NCORES = 8
T_TOTAL = B_ * S
KT = D // 128          # 16 contraction tiles
OC = OUT // 512        # 4 output chunks (one PSUM bank each)
ER = E * R             # 128


def build_nc(T):
    """Build the per-core kernel for T tokens (T % 128 == 0)."""
    n_tt = T // 128
    nc = bacc.Bacc("TRN2", target_bir_lowering=False, debug=False,
                   num_devices=NCORES)

    x_d = nc.dram_tensor("x", [T, D], f32, kind="ExternalInput").ap()
    w_d = nc.dram_tensor("w", [D, OUT], f32r, kind="ExternalInput").ap()
    ra_d = nc.dram_tensor("ra", [D, E + ER], f32, kind="ExternalInput").ap()
    b_d = nc.dram_tensor("b", [ER, OUT], f32r, kind="ExternalInput").ap()
    bias_d = nc.dram_tensor("bias", [128, OUT], f32, kind="ExternalInput").ap()
    ident_d = nc.dram_tensor("ident", [128, 128], f32, kind="ExternalInput").ap()
    out_d = nc.dram_tensor("out", [T, OUT], f32, kind="ExternalOutput").ap()

    QW = 512  # x quarter width

    with tile.TileContext(nc) as tc:
        with (
            tc.tile_pool(name="wpool", bufs=1) as wpool,
            tc.tile_pool(name="consts", bufs=1) as consts,
            tc.tile_pool(name="xin", bufs=6) as xin,
            tc.tile_pool(name="xtr", bufs=2) as xtr,
            tc.tile_pool(name="xtf", bufs=2) as xtf,
            tc.tile_pool(name="mask", bufs=2) as maskp,
            tc.tile_pool(name="outp", bufs=4) as outp,
            tc.tile_pool(name="psb", bufs=1, space="PSUM") as psb,
            tc.tile_pool(name="pst", bufs=2, space="PSUM") as pst,
            tc.tile_pool(name="pssm", bufs=2, space="PSUM") as pssm,
        ):
            ident = consts.tile([128, 128], f32)
            nc.sync.dma_start(ident[:], ident_d[:, :])
            ra_s = consts.tile([128, KT, E + ER], f32)
            b_s = consts.tile([128, OUT], f32r)
            bias_s = consts.tile([128, OUT], f32)
            w_s = wpool.tile([128, KT, OUT], f32r)

            def load_x(tt):
                qs = []
                for q in range(4):
                    xq = xin.tile([128, QW], f32, name="xq")
                    nc.sync.dma_start(
                        xq[:],
                        x_d[tt * 128:(tt + 1) * 128, q * QW:(q + 1) * QW])
                    qs.append(xq)
                return qs

            def load_consts_and_w():
                nc.sync.dma_start(
                    ra_s[:], ra_d.rearrange("(k p) e -> p k e", p=128))
                for k in range(KT):
                    nc.sync.dma_start(w_s[:, k, :],
                                      w_d[k * 128:(k + 1) * 128, :])
                nc.sync.dma_start(bias_s[:], bias_d[:, :])
                nc.sync.dma_start(b_s[:], b_d[:, :])

            # front phase: transposes + [router|A] fp32 matmul + top-1 mask
            def front(tt, quarters):
                xT_r = xtr.tile([128, KT, 128], f32r, name="xT_r")
                xT_f = xtf.tile([128, KT, 128], f32, name="xT_f")
                for g in range(4):
                    pt = pst.tile([128, 512], f32, name="pt")
                    for j in range(4):
                        nc.tensor.transpose(
                            pt[:, j * 128:(j + 1) * 128],
                            quarters[g][:, j * 128:(j + 1) * 128],
                            ident[:],
                        )
                    nc.vector.tensor_copy(xT_r[:, g * 4:(g + 1) * 4, :], pt[:])
                    nc.scalar.copy(xT_f[:, g * 4:(g + 1) * 4, :], pt[:])
                ps_ra = pssm.tile([128, E + ER], f32, name="ps_ra")
                for k in range(KT):
                    nc.tensor.matmul(
                        ps_ra[:], xT_f[:, k, :], ra_s[:, k, :],
                        start=(k == 0), stop=(k == KT - 1),
                    )
                rowmax = maskp.tile([128, 1], f32, name="rowmax")
                nc.vector.tensor_reduce(
                    rowmax[:], ps_ra[:, 0:E], axis=mybir.AxisListType.X,
                    op=mybir.AluOpType.max,
                )
                onehot = maskp.tile([128, E], f32, name="onehot")
                nc.vector.tensor_scalar(
                    onehot[:], ps_ra[:, 0:E], rowmax[:], None,
                    op0=mybir.AluOpType.is_ge,
                )
                hm = maskp.tile([128, ER], f32, name="hm")
                nc.vector.tensor_tensor(
                    hm[:].rearrange("p (e r) -> p e r", e=E),
                    ps_ra[:, E:E + ER].rearrange("p (e r) -> p e r", e=E),
                    onehot[:].unsqueeze(-1).broadcast_to((128, E, R)),
                    op=mybir.AluOpType.mult,
                )
                return xT_r, hm

            def hm_transpose(hm):
                pT = pst.tile([128, 512], f32, name="pt")
                nc.tensor.transpose(pT[:, 0:128], hm[:], ident[:])
                hT_r = maskp.tile([128, 128], f32r, name="hT_r")
                nc.vector.tensor_copy(hT_r[:], pT[:, 0:128])
                return hT_r

            # one output-chunk pass: 16 accumulating base MMs + LoRA delta MM,
            # then bias add (DVE) and the output DMA — frees the PSUM bank.
            def base_oc(tt, xT_r, hT_r, ps_base, oc):
                sl = slice(oc * 512, (oc + 1) * 512)
                for k in range(KT):
                    nc.tensor.matmul(
                        ps_base[:, sl], xT_r[:, k, :], w_s[:, k, sl],
                        start=(k == 0), stop=False,
                    )
                nc.tensor.matmul(
                    ps_base[:, sl], hT_r[:], b_s[:, sl],
                    start=False, stop=True,
                )
                o_s = outp.tile([128, 512], f32, name="o_s")
                nc.vector.tensor_tensor(
                    o_s[:], ps_base[:, sl], bias_s[:, sl],
                    op=mybir.AluOpType.add,
                )
                nc.sync.dma_start(
                    out_d[tt * 128:(tt + 1) * 128, sl], o_s[:])

            xq = [load_x(0), load_x(1)]
            load_consts_and_w()
            prev = None  # (tt, xT_r, hT_r) pending base phase
            for tt in range(n_tt):
                quarters = xq.pop(0)
                xT_r, hm = front(tt, quarters)
                if tt + 2 < n_tt:
                    xq.append(load_x(tt + 2))
                if prev is not None:
                    ptt, pxT, phT = prev
                    ps_base = psb.tile([128, OUT], f32, name="ps_base")
                    base_oc(ptt, pxT, phT, ps_base, 0)
                    hT_cur = hm_transpose(hm)
                    for oc in range(1, OC):
                        base_oc(ptt, pxT, phT, ps_base, oc)
                else:
                    hT_cur = hm_transpose(hm)
                prev = (tt, xT_r, hT_cur)
            ptt, pxT, phT = prev
            ps_base = psb.tile([128, OUT], f32, name="ps_base")
            for oc in range(OC):
                base_oc(ptt, pxT, phT, ps_base, oc)

    nc.compile()
    return nc


_CACHE = {}


def _get_nc(T):
    if T not in _CACHE:
        _CACHE[T] = build_nc(T)
    return _CACHE[T]


def _prep_weights(W_base, b_base, router_W, A, Bw):
    W_baseT = np.ascontiguousarray(W_base.astype(np.float32).T)
    ra = np.concatenate(
        [router_W.astype(np.float32).T,
         A.astype(np.float32).reshape(ER, D).T], axis=1)  # [D, E+ER]
    ra = np.ascontiguousarray(ra)
    B_all = np.ascontiguousarray(
        Bw.astype(np.float32).transpose(0, 2, 1).reshape(ER, OUT) * SCALING)
    bias_rep = np.ascontiguousarray(
        np.broadcast_to(b_base.astype(np.float32), (128, OUT)))
    ident = np.eye(128, dtype=np.float32)
    return W_baseT, ra, B_all, bias_rep, ident


def kernel(x, W_base, b_base, router_W, A, Bw):
    x = np.asarray(x, dtype=np.float32)
    xf = np.ascontiguousarray(x.reshape(T_TOTAL, D))
    W_baseT, ra, B_all, bias_rep, ident = _prep_weights(
        np.asarray(W_base), np.asarray(b_base), np.asarray(router_W),
        np.asarray(A), np.asarray(Bw))

    T = T_TOTAL // NCORES
    nc = _get_nc(T)
    in_maps = []
    for c in range(NCORES):
        in_maps.append({
            "x": xf[c * T:(c + 1) * T],
            "w": W_baseT,
            "ra": ra,
            "b": B_all,
            "bias": bias_rep,
            "ident": ident,
        })
    res = bass_utils.run_bass_kernel_spmd(
        nc, in_maps, core_ids=list(range(NCORES)))
    out = np.concatenate([res.results[c]["out"] for c in range(NCORES)], axis=0)
    return out.reshape(B_, S, OUT)


# revision 11
# speedup vs baseline: 1.0997x; 1.0997x over previous
"""MoLE layer (frozen base linear + top-1 routed LoRA experts) on 8 TRN2 cores.

Strategy: data-parallel over tokens (16384 tokens -> 2048/core), all weights
replicated. Per core, per 128-token tile:
  - DMA x tile [128, 2048], PE-transpose to xT (both an fp32r copy for the
    base matmul and an exact-f32 copy for router logits / LoRA down-proj)
  - base:   psum[t, o]  += xT_r[k].T @ W_baseT[k, o]    (fp32r, N=512, full rate)
  - logits: psum[t, 8]  += xT_f[k].T @ router_WT[k]     (fp32 exact: argmax safety)
  - h:      psum[t, er] += xT_f[k].T @ A_allT[k]        (fp32 exact, er = E*R = 128)
  - mask h by top-1 expert (rowmax + is_ge -> one-hot, broadcast over r)
  - delta:  psum[t, o]  += hT_r.T @ B_all[er, o]        (fp32r; SCALING folded in)
  - out = psum + bias  (DVE), DMA out
No collectives needed.
"""

import numpy as np

import concourse.mybir as mybir
import concourse.tile as tile
from concourse import bacc, bass_utils

f32 = mybir.dt.float32
f32r = mybir.dt.float32r

B_, S, D, OUT, E, R = 4, 4096, 2048, 2048, 8, 16
SCALING = 32.0 / 16.0
NCORES = 8
T_TOTAL = B_ * S
KT = D // 128          # 16 contraction tiles
OC = OUT // 512        # 4 output chunks (one PSUM bank each)
ER = E * R             # 128


def build_nc(T):
    """Build the per-core kernel for T tokens (T % 128 == 0)."""
    n_tt = T // 128
    nc = bacc.Bacc("TRN2", target_bir_lowering=False, debug=False,
                   num_devices=NCORES)

    x_d = nc.dram_tensor("x", [T, D], f32, kind="ExternalInput").ap()
    w_d = nc.dram_tensor("w", [D, OUT], f32r, kind="ExternalInput").ap()
    ra_d = nc.dram_tensor("ra", [D, E + ER], f32, kind="ExternalInput").ap()
    b_d = nc.dram_tensor("b", [ER, OUT], f32r, kind="ExternalInput").ap()
    bias_d = nc.dram_tensor("bias", [128, OUT], f32, kind="ExternalInput").ap()
    ident_d = nc.dram_tensor("ident", [128, 128], f32, kind="ExternalInput").ap()
    out_d = nc.dram_tensor("out", [T, OUT], f32, kind="ExternalOutput").ap()

    QW = 512  # x quarter width

    with tile.TileContext(nc) as tc:
        with (
            tc.tile_pool(name="wpool", bufs=1) as wpool,
            tc.tile_pool(name="consts", bufs=1) as consts,
            tc.tile_pool(name="xin", bufs=5) as xin,
            tc.tile_pool(name="xtr", bufs=3) as xtr,
            tc.tile_pool(name="xtf", bufs=2) as xtf,
            tc.tile_pool(name="mask", bufs=4) as maskp,
            tc.tile_pool(name="outp", bufs=3) as outp,
            tc.tile_pool(name="psb", bufs=1, space="PSUM") as psb,
            tc.tile_pool(name="pst", bufs=2, space="PSUM") as pst,
            tc.tile_pool(name="pssm", bufs=2, space="PSUM") as pssm,
        ):
            ident = consts.tile([128, 128], f32)
            nc.sync.dma_start(ident[:], ident_d[:, :])
            ra_s = consts.tile([128, KT, E + ER], f32)
            b_s = consts.tile([128, OUT], f32r)
            bias_s = consts.tile([128, OUT], f32)
            w_s = wpool.tile([128, KT, OUT], f32r)

            def load_x(tt):
                qs = []
                for q in range(4):
                    xq = xin.tile([128, QW], f32, name="xq")
                    nc.sync.dma_start(
                        xq[:],
                        x_d[tt * 128:(tt + 1) * 128, q * QW:(q + 1) * QW])
                    qs.append(xq)
                return qs

            def load_consts_and_w():
                nc.sync.dma_start(
                    ra_s[:], ra_d.rearrange("(k p) e -> p k e", p=128))
                for k in range(KT):
                    nc.sync.dma_start(w_s[:, k, :],
                                      w_d[k * 128:(k + 1) * 128, :])
                nc.sync.dma_start(bias_s[:], bias_d[:, :])
                nc.sync.dma_start(b_s[:], b_d[:, :])

            # front phase: transposes + [router|A] fp32 matmul + top-1 mask
            def front(tt, quarters):
                xT_r = xtr.tile([128, KT, 128], f32r, name="xT_r")
                ps_ra = pssm.tile([128, E + ER], f32, name="ps_ra")
                for half in range(2):
                    xT_f = xtf.tile([128, KT // 2, 128], f32, name="xT_f")
                    for gg in range(2):
                        g = half * 2 + gg
                        pt = pst.tile([128, 512], f32, name="pt")
                        for j in range(4):
                            nc.tensor.transpose(
                                pt[:, j * 128:(j + 1) * 128],
                                quarters[g][:, j * 128:(j + 1) * 128],
                                ident[:],
                            )
                        nc.vector.tensor_copy(
                            xT_r[:, g * 4:(g + 1) * 4, :], pt[:])
                        nc.scalar.copy(
                            xT_f[:, gg * 4:(gg + 1) * 4, :], pt[:])
                    for kk in range(KT // 2):
                        k = half * 8 + kk
                        nc.tensor.matmul(
                            ps_ra[:], xT_f[:, kk, :], ra_s[:, k, :],
                            start=(k == 0), stop=(k == KT - 1),
                        )
                rowmax = maskp.tile([128, 1], f32, name="rowmax")
                nc.vector.tensor_reduce(
                    rowmax[:], ps_ra[:, 0:E], axis=mybir.AxisListType.X,
                    op=mybir.AluOpType.max,
                )
                onehot = maskp.tile([128, E], f32, name="onehot")
                nc.vector.tensor_scalar(
                    onehot[:], ps_ra[:, 0:E], rowmax[:], None,
                    op0=mybir.AluOpType.is_ge,
                )
                hm = maskp.tile([128, ER], f32, name="hm")
                nc.vector.tensor_tensor(
                    hm[:].rearrange("p (e r) -> p e r", e=E),
                    ps_ra[:, E:E + ER].rearrange("p (e r) -> p e r", e=E),
                    onehot[:].unsqueeze(-1).broadcast_to((128, E, R)),
                    op=mybir.AluOpType.mult,
                )
                return xT_r, hm

            def hm_transpose(hm):
                pT = pst.tile([128, 512], f32, name="pt")
                nc.tensor.transpose(pT[:, 0:128], hm[:], ident[:])
                hT_r = maskp.tile([128, 128], f32r, name="hT_r")
                nc.vector.tensor_copy(hT_r[:], pT[:, 0:128])
                return hT_r

            # one output-chunk pass: 16 accumulating base MMs + LoRA delta MM,
            # then bias add (DVE) and the output DMA — frees the PSUM bank.
            def base_oc(tt, xT_r, hT_r, ps_base, oc, mid=None, k0=0, k1=KT):
                sl = slice(oc * 512, (oc + 1) * 512)
                for k in range(k0, k1):
                    nc.tensor.matmul(
                        ps_base[:, sl], xT_r[:, k, :], w_s[:, k, sl],
                        start=(k == 0), stop=False,
                    )
                if k1 < KT:
                    return
                nc.tensor.matmul(
                    ps_base[:, sl], hT_r[:], b_s[:, sl],
                    start=False, stop=True,
                )
                o_s = outp.tile([128, 512], f32, name="o_s")
                nc.vector.tensor_tensor(
                    o_s[:], ps_base[:, sl], bias_s[:, sl],
                    op=mybir.AluOpType.add,
                )
                nc.sync.dma_start(
                    out_d[tt * 128:(tt + 1) * 128, sl], o_s[:])

            xq = [load_x(0), load_x(1)]
            load_consts_and_w()
            prev = None  # (tt, xT_r, hT_r) pending base phase
            for tt in range(n_tt):
                quarters = xq.pop(0)
                xT_r, hm = front(tt, quarters)
                if tt + 2 < n_tt:
                    xq.append(load_x(tt + 2))
                if prev is not None:
                    ptt, pxT, phT = prev
                    ps_base = psb.tile([128, OUT], f32, name="ps_base")
                    base_oc(ptt, pxT, phT, ps_base, 0, k0=0, k1=3)
                    hT_cur = hm_transpose(hm)
                    base_oc(ptt, pxT, phT, ps_base, 0, k0=3, k1=KT)
                    for oc in range(1, OC):
                        base_oc(ptt, pxT, phT, ps_base, oc)
                else:
                    hT_cur = hm_transpose(hm)
                prev = (tt, xT_r, hT_cur)
            ptt, pxT, phT = prev
            ps_base = psb.tile([128, OUT], f32, name="ps_base")
            for oc in range(OC):
                base_oc(ptt, pxT, phT, ps_base, oc)

    nc.compile()
    return nc


_CACHE = {}


def _get_nc(T):
    if T not in _CACHE:
        _CACHE[T] = build_nc(T)
    return _CACHE[T]


def _prep_weights(W_base, b_base, router_W, A, Bw):
    W_baseT = np.ascontiguousarray(W_base.astype(np.float32).T)
    ra = np.concatenate(
        [router_W.astype(np.float32).T,
         A.astype(np.float32).reshape(ER, D).T], axis=1)  # [D, E+ER]
    ra = np.ascontiguousarray(ra)
    B_all = np.ascontiguousarray(
        Bw.astype(np.float32).transpose(0, 2, 1).reshape(ER, OUT) * SCALING)
    bias_rep = np.ascontiguousarray(
        np.broadcast_to(b_base.astype(np.float32), (128, OUT)))
    ident = np.eye(128, dtype=np.float32)
    return W_baseT, ra, B_all, bias_rep, ident


def kernel(x, W_base, b_base, router_W, A, Bw):
    x = np.asarray(x, dtype=np.float32)
    xf = np.ascontiguousarray(x.reshape(T_TOTAL, D))
    W_baseT, ra, B_all, bias_rep, ident = _prep_weights(
        np.asarray(W_base), np.asarray(b_base), np.asarray(router_W),
        np.asarray(A), np.asarray(Bw))

    T = T_TOTAL // NCORES
    nc = _get_nc(T)
    in_maps = []
    for c in range(NCORES):
        in_maps.append({
            "x": xf[c * T:(c + 1) * T],
            "w": W_baseT,
            "ra": ra,
            "b": B_all,
            "bias": bias_rep,
            "ident": ident,
        })
    res = bass_utils.run_bass_kernel_spmd(
        nc, in_maps, core_ids=list(range(NCORES)))
    out = np.concatenate([res.results[c]["out"] for c in range(NCORES)], axis=0)
    return out.reshape(B_, S, OUT)


# revision 12
# speedup vs baseline: 1.1087x; 1.0082x over previous
"""MoLE layer (frozen base linear + top-1 routed LoRA experts) on 8 TRN2 cores.

Strategy: data-parallel over tokens (16384 tokens -> 2048/core), all weights
replicated, no collectives. Per core, per 128-token tile (software-pipelined:
the "front" phase of tile t runs while tile t-1's base matmuls execute):
  front: DMA x tile [128, 2048] in quarters, PE-transpose to xT
         (an fp32r-typed copy for the base matmul + an exact-f32 copy),
         ps_ra[t, 0:136] += xT_f[k].T @ [router_WT | A_allT][k]  (fp32 exact
         2-pass matmul -> exact argmax, exact LoRA h), then top-1 mask
         (rowmax + is_ge -> one-hot, broadcast over r) and PE-transpose the
         masked h to hT_r.
  base (oc-outer, one PSUM bank per 512-wide output chunk):
         psum[t, oc] += xT_r[k].T @ W_baseT[k, oc]   (fp32r = full PE rate)
         psum[t, oc] += hT_r.T @ B_all[er, oc]       (LoRA delta; SCALING
         folded into B_all), out = psum + bias (DVE), DMA out -- per chunk,
         so banks free progressively and the next tile's matmuls overlap.
"""

import numpy as np

import concourse.mybir as mybir
import concourse.tile as tile
from concourse import bacc, bass_utils

f32 = mybir.dt.float32
f32r = mybir.dt.float32r

B_, S, D, OUT, E, R = 4, 4096, 2048, 2048, 8, 16
SCALING = 32.0 / 16.0
NCORES = 8
T_TOTAL = B_ * S
KT = D // 128          # 16 contraction tiles
OC = OUT // 512        # 4 output chunks (one PSUM bank each)
ER = E * R             # 128


def build_nc(T):
    """Build the per-core kernel for T tokens (T % 128 == 0)."""
    n_tt = T // 128
    nc = bacc.Bacc("TRN2", target_bir_lowering=False, debug=False,
                   num_devices=NCORES)

    x_d = nc.dram_tensor("x", [T, D], f32, kind="ExternalInput").ap()
    w_d = nc.dram_tensor("w", [D, OUT], f32r, kind="ExternalInput").ap()
    ra_d = nc.dram_tensor("ra", [D, E + ER], f32, kind="ExternalInput").ap()
    b_d = nc.dram_tensor("b", [ER, OUT], f32r, kind="ExternalInput").ap()
    bias_d = nc.dram_tensor("bias", [128, OUT], f32, kind="ExternalInput").ap()
    ident_d = nc.dram_tensor("ident", [128, 128], f32, kind="ExternalInput").ap()
    out_d = nc.dram_tensor("out", [T, OUT], f32, kind="ExternalOutput").ap()

    QW = 512  # x quarter width

    with tile.TileContext(nc) as tc:
        with (
            tc.tile_pool(name="wpool", bufs=1) as wpool,
            tc.tile_pool(name="consts", bufs=1) as consts,
            tc.tile_pool(name="xin", bufs=5) as xin,
            tc.tile_pool(name="xtr", bufs=3) as xtr,
            tc.tile_pool(name="xtf", bufs=2) as xtf,
            tc.tile_pool(name="mask", bufs=4) as maskp,
            tc.tile_pool(name="outp", bufs=3) as outp,
            tc.tile_pool(name="psb", bufs=1, space="PSUM") as psb,
            tc.tile_pool(name="pst", bufs=2, space="PSUM") as pst,
            tc.tile_pool(name="pssm", bufs=2, space="PSUM") as pssm,
        ):
            ident = consts.tile([128, 128], f32)
            nc.sync.dma_start(ident[:], ident_d[:, :])
            ra_s = consts.tile([128, KT, E + ER], f32)
            b_s = consts.tile([128, OUT], f32r)
            bias_s = consts.tile([128, OUT], f32)
            w_s = wpool.tile([128, KT, OUT], f32r)

            def load_x(tt):
                qs = []
                for q in range(4):
                    xq = xin.tile([128, QW], f32, name="xq")
                    nc.sync.dma_start(
                        xq[:],
                        x_d[tt * 128:(tt + 1) * 128, q * QW:(q + 1) * QW])
                    qs.append(xq)
                return qs

            def load_consts_and_w():
                nc.sync.dma_start(
                    ra_s[:], ra_d.rearrange("(k p) e -> p k e", p=128))
                for k in range(KT):
                    nc.sync.dma_start(w_s[:, k, :],
                                      w_d[k * 128:(k + 1) * 128, :])
                nc.sync.dma_start(bias_s[:], bias_d[:, :])
                nc.sync.dma_start(b_s[:], b_d[:, :])

            # front phase: transposes + [router|A] fp32 matmul + top-1 mask
            def front(tt, quarters):
                xT_r = xtr.tile([128, KT, 128], f32r, name="xT_r")
                ps_ra = pssm.tile([128, E + ER], f32, name="ps_ra")
                for half in range(2):
                    xT_f = xtf.tile([128, KT // 2, 128], f32, name="xT_f")
                    for gg in range(2):
                        g = half * 2 + gg
                        pt = pst.tile([128, 512], f32, name="pt")
                        for j in range(4):
                            nc.tensor.transpose(
                                pt[:, j * 128:(j + 1) * 128],
                                quarters[g][:, j * 128:(j + 1) * 128],
                                ident[:],
                            )
                        nc.vector.tensor_copy(
                            xT_r[:, g * 4:(g + 1) * 4, :], pt[:])
                        nc.scalar.copy(
                            xT_f[:, gg * 4:(gg + 1) * 4, :], pt[:])
                    for kk in range(KT // 2):
                        k = half * 8 + kk
                        nc.tensor.matmul(
                            ps_ra[:], xT_f[:, kk, :], ra_s[:, k, :],
                            start=(k == 0), stop=(k == KT - 1),
                        )
                rowmax = maskp.tile([128, 1], f32, name="rowmax")
                nc.vector.tensor_reduce(
                    rowmax[:], ps_ra[:, 0:E], axis=mybir.AxisListType.X,
                    op=mybir.AluOpType.max,
                )
                onehot = maskp.tile([128, E], f32, name="onehot")
                nc.vector.tensor_scalar(
                    onehot[:], ps_ra[:, 0:E], rowmax[:], None,
                    op0=mybir.AluOpType.is_ge,
                )
                hm = maskp.tile([128, ER], f32, name="hm")
                nc.vector.tensor_tensor(
                    hm[:].rearrange("p (e r) -> p e r", e=E),
                    ps_ra[:, E:E + ER].rearrange("p (e r) -> p e r", e=E),
                    onehot[:].unsqueeze(-1).broadcast_to((128, E, R)),
                    op=mybir.AluOpType.mult,
                )
                return xT_r, hm

            def hm_transpose(hm):
                pT = pst.tile([128, 512], f32, name="pt")
                nc.tensor.transpose(pT[:, 0:128], hm[:], ident[:])
                hT_r = maskp.tile([128, 128], f32r, name="hT_r")
                nc.vector.tensor_copy(hT_r[:], pT[:, 0:128])
                return hT_r

            # one output-chunk pass: 16 accumulating base MMs + LoRA delta MM,
            # then bias add (DVE) and the output DMA — frees the PSUM bank.
            def base_oc(tt, xT_r, hT_r, ps_base, oc, k0=0, k1=KT):
                sl = slice(oc * 512, (oc + 1) * 512)
                for k in range(k0, k1):
                    nc.tensor.matmul(
                        ps_base[:, sl], xT_r[:, k, :], w_s[:, k, sl],
                        start=(k == 0), stop=False,
                    )
                if k1 < KT:
                    return
                nc.tensor.matmul(
                    ps_base[:, sl], hT_r[:], b_s[:, sl],
                    start=False, stop=True,
                )
                o_s = outp.tile([128, 512], f32, name="o_s")
                nc.vector.tensor_tensor(
                    o_s[:], ps_base[:, sl], bias_s[:, sl],
                    op=mybir.AluOpType.add,
                )
                nc.sync.dma_start(
                    out_d[tt * 128:(tt + 1) * 128, sl], o_s[:])

            xq = [load_x(0), load_x(1)]
            load_consts_and_w()
            prev = None  # (tt, xT_r, hT_r) pending base phase
            for tt in range(n_tt):
                quarters = xq.pop(0)
                xT_r, hm = front(tt, quarters)
                if tt + 2 < n_tt:
                    xq.append(load_x(tt + 2))
                if prev is not None:
                    ptt, pxT, phT = prev
                    ps_base = psb.tile([128, OUT], f32, name="ps_base")
                    base_oc(ptt, pxT, phT, ps_base, 0, k0=0, k1=3)
                    hT_cur = hm_transpose(hm)
                    base_oc(ptt, pxT, phT, ps_base, 0, k0=3, k1=KT)
                    for oc in range(1, OC):
                        base_oc(ptt, pxT, phT, ps_base, oc)
                else:
                    hT_cur = hm_transpose(hm)
                prev = (tt, xT_r, hT_cur)
            ptt, pxT, phT = prev
            ps_base = psb.tile([128, OUT], f32, name="ps_base")
            for oc in range(OC):
                base_oc(ptt, pxT, phT, ps_base, oc)

    nc.compile()
    return nc


_CACHE = {}


def _get_nc(T):
    if T not in _CACHE:
        _CACHE[T] = build_nc(T)
    return _CACHE[T]


def _prep_weights(W_base, b_base, router_W, A, Bw):
    W_baseT = np.ascontiguousarray(W_base.astype(np.float32).T)
    ra = np.concatenate(
        [router_W.astype(np.float32).T,
         A.astype(np.float32).reshape(ER, D).T], axis=1)  # [D, E+ER]
    ra = np.ascontiguousarray(ra)
    B_all = np.ascontiguousarray(
        Bw.astype(np.float32).transpose(0, 2, 1).reshape(ER, OUT) * SCALING)
    bias_rep = np.ascontiguousarray(
        np.broadcast_to(b_base.astype(np.float32), (128, OUT)))
    ident = np.eye(128, dtype=np.float32)
    return W_baseT, ra, B_all, bias_rep, ident


def kernel(x, W_base, b_base, router_W, A, Bw):
    x = np.asarray(x, dtype=np.float32)
    xf = np.ascontiguousarray(x.reshape(T_TOTAL, D))
    W_baseT, ra, B_all, bias_rep, ident = _prep_weights(
        np.asarray(W_base), np.asarray(b_base), np.asarray(router_W),
        np.asarray(A), np.asarray(Bw))

    T = T_TOTAL // NCORES
    nc = _get_nc(T)
    in_maps = []
    for c in range(NCORES):
        in_maps.append({
            "x": xf[c * T:(c + 1) * T],
            "w": W_baseT,
            "ra": ra,
            "b": B_all,
            "bias": bias_rep,
            "ident": ident,
        })
    res = bass_utils.run_bass_kernel_spmd(
        nc, in_maps, core_ids=list(range(NCORES)))
    out = np.concatenate([res.results[c]["out"] for c in range(NCORES)], axis=0)
    return out.reshape(B_, S, OUT)


# revision 15
# speedup vs baseline: 1.1209x; 1.0110x over previous
"""MoLE layer (frozen base linear + top-1 routed LoRA experts) on 8 TRN2 cores.

Strategy: data-parallel over tokens (16384 tokens -> 2048/core), all weights
replicated, no collectives. Per core, per 128-token tile (software-pipelined:
the "front" phase of tile t runs while tile t-1's base matmuls execute):
  front: DMA x tile [128, 2048] in quarters, PE-transpose to xT
         (an fp32r-typed copy for the base matmul + an exact-f32 copy),
         ps_ra[t, 0:136] += xT_f[k].T @ [router_WT | A_allT][k]  (fp32 exact
         2-pass matmul -> exact argmax, exact LoRA h), then top-1 mask
         (rowmax + is_ge -> one-hot, broadcast over r) and PE-transpose the
         masked h to hT_r.
  base (oc-outer, one PSUM bank per 512-wide output chunk):
         psum[t, oc] += xT_r[k].T @ W_baseT[k, oc]   (fp32r = full PE rate)
         psum[t, oc] += hT_r.T @ B_all[er, oc]       (LoRA delta; SCALING
         folded into B_all), out = psum + bias (DVE), DMA out -- per chunk,
         so banks free progressively and the next tile's matmuls overlap.
"""

import numpy as np

import concourse.mybir as mybir
import concourse.tile as tile
from concourse import bacc, bass_utils

f32 = mybir.dt.float32
f32r = mybir.dt.float32r

B_, S, D, OUT, E, R = 4, 4096, 2048, 2048, 8, 16
SCALING = 32.0 / 16.0
NCORES = 8
T_TOTAL = B_ * S
KT = D // 128          # 16 contraction tiles
OC = OUT // 512        # 4 output chunks (one PSUM bank each)
ER = E * R             # 128


def build_nc(T):
    """Build the per-core kernel for T tokens (T % 128 == 0)."""
    n_tt = T // 128
    nc = bacc.Bacc("TRN2", target_bir_lowering=False, debug=False,
                   num_devices=NCORES)

    x_d = nc.dram_tensor("x", [T, D], f32, kind="ExternalInput").ap()
    w_d = nc.dram_tensor("w", [D, OUT], f32r, kind="ExternalInput").ap()
    ra_d = nc.dram_tensor("ra", [D, E + ER], f32, kind="ExternalInput").ap()
    b_d = nc.dram_tensor("b", [ER, OUT], f32r, kind="ExternalInput").ap()
    bias_d = nc.dram_tensor("bias", [128, OUT], f32, kind="ExternalInput").ap()
    ident_d = nc.dram_tensor("ident", [128, 128], f32, kind="ExternalInput").ap()
    out_d = nc.dram_tensor("out", [T, OUT], f32, kind="ExternalOutput").ap()

    QW = 512  # x quarter width

    with tile.TileContext(nc) as tc:
        with (
            tc.tile_pool(name="wpool", bufs=1) as wpool,
            tc.tile_pool(name="consts", bufs=1) as consts,
            tc.tile_pool(name="xin", bufs=5) as xin,
            tc.tile_pool(name="xtr", bufs=3) as xtr,
            tc.tile_pool(name="xtf", bufs=2) as xtf,
            tc.tile_pool(name="mask", bufs=4) as maskp,
            tc.tile_pool(name="outp", bufs=3) as outp,
            tc.tile_pool(name="psb", bufs=1, space="PSUM") as psb,
            tc.tile_pool(name="pst", bufs=2, space="PSUM") as pst,
            tc.tile_pool(name="pssm", bufs=2, space="PSUM") as pssm,
        ):
            ident = consts.tile([128, 128], f32)
            nc.sync.dma_start(ident[:], ident_d[:, :])
            ra_s = consts.tile([128, KT, E + ER], f32)
            b_s = consts.tile([128, OUT], f32r)
            bias_s = consts.tile([128, OUT], f32)
            w_s = wpool.tile([128, KT, OUT], f32r)

            def load_x(tt):
                qs = []
                for q in range(4):
                    xq = xin.tile([128, QW], f32, name="xq")
                    nc.sync.dma_start(
                        xq[:],
                        x_d[tt * 128:(tt + 1) * 128, q * QW:(q + 1) * QW])
                    qs.append(xq)
                return qs

            def load_consts_and_w():
                nc.sync.dma_start(
                    ra_s[:], ra_d.rearrange("(k p) e -> p k e", p=128))
                for k in range(KT):
                    nc.sync.dma_start(w_s[:, k, :],
                                      w_d[k * 128:(k + 1) * 128, :])
                nc.sync.dma_start(bias_s[:], bias_d[:, :])
                nc.sync.dma_start(b_s[:], b_d[:, :])

            # front phase: transposes + [router|A] fp32 matmul + top-1 mask
            def front(tt, quarters):
                xT_r = xtr.tile([128, KT, 128], f32r, name="xT_r")
                ps_ra = pssm.tile([128, E + ER], f32, name="ps_ra")
                for half in range(2):
                    xT_f = xtf.tile([128, KT // 2, 128], f32, name="xT_f")
                    for gg in range(2):
                        g = half * 2 + gg
                        pt = pst.tile([128, 512], f32, name="pt")
                        for j in range(4):
                            nc.tensor.transpose(
                                pt[:, j * 128:(j + 1) * 128],
                                quarters[g][:, j * 128:(j + 1) * 128],
                                ident[:],
                            )
                        nc.vector.tensor_copy(
                            xT_r[:, g * 4:(g + 1) * 4, :], pt[:])
                        nc.scalar.copy(
                            xT_f[:, gg * 4:(gg + 1) * 4, :], pt[:])
                    for kk in range(KT // 2):
                        k = half * 8 + kk
                        nc.tensor.matmul(
                            ps_ra[:], xT_f[:, kk, :], ra_s[:, k, :],
                            start=(k == 0), stop=(k == KT - 1),
                        )
                rowmax = maskp.tile([128, 1], f32, name="rowmax")
                nc.vector.tensor_reduce(
                    rowmax[:], ps_ra[:, 0:E], axis=mybir.AxisListType.X,
                    op=mybir.AluOpType.max,
                )
                onehot = maskp.tile([128, E], f32, name="onehot")
                nc.vector.tensor_scalar(
                    onehot[:], ps_ra[:, 0:E], rowmax[:], None,
                    op0=mybir.AluOpType.is_ge,
                )
                hm = maskp.tile([128, ER], f32, name="hm")
                nc.vector.tensor_tensor(
                    hm[:].rearrange("p (e r) -> p e r", e=E),
                    ps_ra[:, E:E + ER].rearrange("p (e r) -> p e r", e=E),
                    onehot[:].unsqueeze(-1).broadcast_to((128, E, R)),
                    op=mybir.AluOpType.mult,
                )
                return xT_r, hm

            def hm_transpose(hm):
                pT = pst.tile([128, 512], f32, name="pt")
                nc.tensor.transpose(pT[:, 0:128], hm[:], ident[:])
                hT_r = maskp.tile([128, 128], f32r, name="hT_r")
                nc.vector.tensor_copy(hT_r[:], pT[:, 0:128])
                return hT_r

            # one output-chunk pass: 16 accumulating base MMs + LoRA delta MM,
            # then bias add (DVE) and the output DMA — frees the PSUM bank.
            def base_oc(tt, xT_r, hT_r, ps_base, oc, k0=0, k1=KT):
                sl = slice(oc * 512, (oc + 1) * 512)
                for k in range(k0, k1):
                    nc.tensor.matmul(
                        ps_base[:, sl], xT_r[:, k, :], w_s[:, k, sl],
                        start=(k == 0), stop=False,
                    )
                if k1 < KT:
                    return
                nc.tensor.matmul(
                    ps_base[:, sl], hT_r[:], b_s[:, sl],
                    start=False, stop=True,
                )
                o_s = outp.tile([128, 512], f32, name="o_s")
                nc.vector.tensor_tensor(
                    o_s[:], ps_base[:, sl], bias_s[:, sl],
                    op=mybir.AluOpType.add,
                )
                nc.sync.dma_start(
                    out_d[tt * 128:(tt + 1) * 128, sl], o_s[:])

            xq = [load_x(0), load_x(1)]
            load_consts_and_w()
            prev = None  # (tt, xT_r, hT_r) pending base phase
            for tt in range(n_tt):
                quarters = xq.pop(0)
                xT_r, hm = front(tt, quarters)
                if tt + 2 < n_tt:
                    xq.append(load_x(tt + 2))
                if prev is not None:
                    ptt, pxT, phT = prev
                    ps_base = psb.tile([128, OUT], f32, name="ps_base")
                    base_oc(ptt, pxT, phT, ps_base, 0, k0=0, k1=3)
                    hT_cur = hm_transpose(hm)
                    base_oc(ptt, pxT, phT, ps_base, 0, k0=3, k1=KT)
                    for oc in range(1, OC):
                        base_oc(ptt, pxT, phT, ps_base, oc)
                else:
                    hT_cur = hm_transpose(hm)
                prev = (tt, xT_r, hT_cur)
            ptt, pxT, phT = prev
            ps_base = psb.tile([128, OUT], f32, name="ps_base")
            for oc in range(OC):
                base_oc(ptt, pxT, phT, ps_base, oc)

    nc.compile()
    return nc


_CACHE = {}


def _get_nc(T):
    if T not in _CACHE:
        _CACHE[T] = build_nc(T)
    return _CACHE[T]


def _prep_weights(W_base, b_base, router_W, A, Bw):
    W_baseT = np.ascontiguousarray(W_base.astype(np.float32).T)
    ra = np.concatenate(
        [router_W.astype(np.float32).T,
         A.astype(np.float32).reshape(ER, D).T], axis=1)  # [D, E+ER]
    ra = np.ascontiguousarray(ra)
    B_all = np.ascontiguousarray(
        Bw.astype(np.float32).transpose(0, 2, 1).reshape(ER, OUT) * SCALING)
    bias_rep = np.ascontiguousarray(
        np.broadcast_to(b_base.astype(np.float32), (128, OUT)))
    ident = np.eye(128, dtype=np.float32)
    return W_baseT, ra, B_all, bias_rep, ident


def kernel(x, W_base, b_base, router_W, A, Bw):
    x = np.asarray(x, dtype=np.float32)
    xf = np.ascontiguousarray(x.reshape(T_TOTAL, D))
    W_baseT, ra, B_all, bias_rep, ident = _prep_weights(
        np.asarray(W_base), np.asarray(b_base), np.asarray(router_W),
        np.asarray(A), np.asarray(Bw))

    T = T_TOTAL // NCORES
    nc = _get_nc(T)
    in_maps = []
    for c in range(NCORES):
        in_maps.append({
            "x": xf[c * T:(c + 1) * T],
            "w": W_baseT,
            "ra": ra,
            "b": B_all,
            "bias": bias_rep,
            "ident": ident,
        })
    res = bass_utils.run_bass_kernel_spmd(
        nc, in_maps, core_ids=list(range(NCORES)))
    out = np.concatenate([res.results[c]["out"] for c in range(NCORES)], axis=0)
    return out.reshape(B_, S, OUT)
